# revision 10
# baseline (speedup 1.0000x reference)
"""Trainium2 Bass kernel for nn_DBTKT_84164179132922 (dense_transformer).

Contract: kernel(**inputs) takes the FULL unsharded inputs (as produced by
setup_inputs()) and returns the full (output_backward, for_correlation_weight)
tuple. Internally shards the batch dim (32) across 8 NeuronCores (4 each).

Math (per batch b):
  qh   = q @ Wq + bq                          [512]
  u    = per-head fold of Wk with qh          [1024, 4]   (host, 0.04% of FLOPs)
  sc   = (k @ u + bk.qh) / sqrt(128)          [4, 1024]   (device)
  qs   = sc @ qe.T                            [4, 2048]
  fc   = qs[0]                                output 2
  g    = sigmoid(lam*cw + (1-lam)*qs)         [4, 2048]
  vhT  = Wv.T @ v.T  (+bv)                    [512, 2048]
  out  = (g ⊙ vhT).T-proj: gatedT.T @ Wo + bo [2048, 512]  output 1
"""

import sys

sys.path.insert(0, "/opt/trn_rl_repo")

import numpy as np

B, LK, N, D, H, DK = 32, 1024, 2048, 512, 4, 128
F2, F3 = 2 * D, 3 * D  # 1024, 1536
NCORES = 8
NB = B // NCORES  # batches per core
NQ = 4  # seq quarters of 512
QS = N // NQ  # 512

_cache = {}


def _patch_tile_drain():
    """This walrus build caps sync waits at 1 per non-EventSemaphore
    instruction; Tile's final drain can carry several. Split the extra waits
    onto bare drain instructions (same semantics: all waits happen before the
    end-of-kernel barrier + semaphore reset)."""
    import concourse.tile as tile
    import concourse.mybir as mybir
    from bass_rust import ScopedClock

    if getattr(tile.TileContext, "_drain_patch_applied", False):
        return

    def _drain_and_barrier(self, tick_clock, wait_clock):
        nc_ = self.nc
        drain_inst = nc_.sync.drain()
        wait_clock.add_sem_waits(
            drain_inst.ins, ScopedClock({None: tick_clock.global_clock})
        )
        si = drain_inst.ins.sync_info
        if si is not None and si.on_wait is not None and len(si.on_wait) > 1:
            waits = list(si.on_wait)
            drain_inst.ins.sync_info = mybir.SyncInfo(
                on_wait=waits[:1], on_update=list(si.on_update or [])
            )
            for w in waits[1:]:
                extra = nc_.sync.drain()
                esi = extra.ins.sync_info
                extra.ins.sync_info = mybir.SyncInfo(
                    on_wait=[w],
                    on_update=list(esi.on_update or []) if esi is not None else [],
                )
        nc_.all_engine_barrier()
        assert self.sems is not None
        popped = nc_._tile_sem_poison_stack.pop()
        assert popped is self._sem_poison
        nc_.clear_and_free_semaphores(list(self.sems.allocated().values()))
        nc_.all_engine_barrier()

    tile.TileContext._drain_and_barrier = _drain_and_barrier
    tile.TileContext._drain_patch_applied = True


def _build_module(nb=NB):
    import concourse.bass as bass
    import concourse.mybir as mybir
    import concourse.tile as tile
    from concourse import bacc, masks

    _patch_tile_drain()

    f32 = mybir.dt.float32
    bf16 = mybir.dt.bfloat16
    SQ = float(1.0 / np.sqrt(DK))

    nc = bacc.Bacc("TRN2", target_bir_lowering=False, debug=False,
                   num_devices=NCORES)

    # ---- DRAM I/O ----
    k_h = nc.declare_dram_parameter("k", [nb, LK, F2], f32, isOutput=False)
    v_h = nc.declare_dram_parameter("v", [nb, N, F3], f32, isOutput=False)
    qe_h = nc.declare_dram_parameter("qe", [nb, N, LK], f32, isOutput=False)
    cwl_h = nc.declare_dram_parameter("cwl", [nb, H, N], f32, isOutput=False)
    u_h = nc.declare_dram_parameter("u", [nb, F2, H], f32, isOutput=False)
    c_h = nc.declare_dram_parameter("c", [nb, H], f32, isOutput=False)
    wv_h = nc.declare_dram_parameter("wv", [F3, D], f32, isOutput=False)
    wo_h = nc.declare_dram_parameter("wo", [D, D], f32, isOutput=False)
    bv_h = nc.declare_dram_parameter("bv", [D], f32, isOutput=False)
    bo_h = nc.declare_dram_parameter("bo", [D], f32, isOutput=False)
    lam1_h = nc.declare_dram_parameter("lam1", [H], f32, isOutput=False)
    sel_h = nc.declare_dram_parameter("sel", [H, D], f32, isOutput=False)

    out_h = nc.declare_dram_parameter("out", [nb, N, D], f32, isOutput=True)
    fc_h = nc.declare_dram_parameter("fc", [nb, N], f32, isOutput=True)

    FCK = F2 // 128  # 8 feature chunks for k
    FCV = F3 // 128  # 12 feature chunks for v
    LT = LK // 128   # 8 lk tiles
    MD = D // 128    # 4 dmodel chunks (== heads)

    with tile.TileContext(nc) as tc:
        with (
            tc.tile_pool(name="consts", bufs=1) as consts,
            tc.tile_pool(name="kpool", bufs=2) as kpool,
            tc.tile_pool(name="qepool", bufs=2) as qepool,
            tc.tile_pool(name="vpool", bufs=2) as vpool,
            tc.tile_pool(name="ktc", bufs=2) as ktc_pool,
            tc.tile_pool(name="chunk", bufs=3) as chunk_pool,
            tc.tile_pool(name="gate", bufs=2) as gate_pool,
            tc.tile_pool(name="gated", bufs=2) as gated_pool,
            tc.tile_pool(name="outst", bufs=2) as outst_pool,
            tc.tile_pool(name="scp", bufs=2) as scp_pool,
            tc.tile_pool(name="gp", bufs=2) as gp_pool,
            tc.tile_pool(name="fcp", bufs=2) as fcp_pool,
            tc.tile_pool(name="ps_stage", bufs=2, space="PSUM") as ps_stage,
            tc.tile_pool(name="ps_vh", bufs=1, space="PSUM") as ps_vh,
            tc.tile_pool(name="ps_oqs", bufs=2, space="PSUM") as ps_oqs,
        ):
            # ---- constants / weights (once) ----
            ident = consts.tile([128, 128], bf16)
            masks.make_identity(nc, ident[:, :])

            wv_s = consts.tile([128, FCV, D], bf16)
            nc.gpsimd.dma_start(
                out=wv_s[:, :, :],
                in_=wv_h.rearrange("(c p) d -> p c d", p=128),
            )
            wo_s = consts.tile([128, MD, D], bf16)
            nc.gpsimd.dma_start(
                out=wo_s[:, :, :],
                in_=wo_h.rearrange("(c p) d -> p c d", p=128),
            )
            u_s = consts.tile([128, nb, FCK, H], bf16)
            nc.gpsimd.dma_start(
                out=u_s[:, :, :, :],
                in_=u_h.rearrange("b (c p) h -> p b c h", p=128),
            )
            c_s = consts.tile([H, nb], f32)
            nc.gpsimd.dma_start(out=c_s[:, :], in_=c_h.rearrange("b h -> h b"))
            bv_s = consts.tile([128, MD], f32)
            nc.gpsimd.dma_start(
                out=bv_s[:, :], in_=bv_h.rearrange("(m p) -> p m", p=128)
            )
            bo_rep = consts.tile([128, D], f32)
            bo_ap = bo_h.ap()
            nc.gpsimd.dma_start(
                out=bo_rep[:, :],
                in_=bass.AP(tensor=bo_ap.tensor, offset=bo_ap.offset,
                            ap=[[0, 128]] + list(bo_ap.ap)),
            )
            lam1_s = consts.tile([H, 1], f32)
            nc.gpsimd.dma_start(out=lam1_s[:, :],
                                in_=lam1_h.rearrange("(h o) -> h o", o=1))
            sel_s = consts.tile([H, D], bf16)
            nc.gpsimd.dma_start(out=sel_s[:, :], in_=sel_h[:, :])

            def _copy(i, dst, src):
                if i % 2 == 0:
                    nc.vector.tensor_copy(dst, src)
                else:
                    nc.scalar.copy(out=dst, in_=src)

            for b in range(nb):
                # ---------- scores: sc[4, LK] = (U.T @ kT) * SQ + c ----------
                k_nat = kpool.tile([128, LT, F2], bf16, tag="k_nat")
                nc.gpsimd.dma_start(
                    out=k_nat[:, :, :],
                    in_=k_h[b].rearrange("(t p) f -> p t f", p=128),
                )
                ps_sc = ps_vh.tile([4, LK], f32, tag="vh")
                for fc in range(FCK):
                    st_k = ps_stage.tile([128, LK], bf16, tag="stage")
                    for lt in range(LT):
                        nc.tensor.transpose(
                            st_k[:, lt * 128:(lt + 1) * 128],
                            k_nat[:, lt, fc * 128:(fc + 1) * 128],
                            ident[:, :],
                        )
                    kt_c = ktc_pool.tile([128, LK], bf16, tag="ktc")
                    _copy(fc, kt_c[:, :], st_k[:, :])
                    for half in range(2):
                        nc.tensor.matmul(
                            ps_sc[:, half * 512:(half + 1) * 512],
                            u_s[:, b, fc, :],
                            kt_c[:, half * 512:(half + 1) * 512],
                            start=(fc == 0), stop=(fc == FCK - 1),
                        )
                # scale + bias -> bf16 scores
                sc_s = scp_pool.tile([4, LK], bf16, tag="sc")
                nc.vector.tensor_scalar(
                    out=sc_s[:, :], in0=ps_sc[:, :],
                    scalar1=SQ, scalar2=c_s[0:4, b:b + 1],
                    op0=mybir.AluOpType.mult, op1=mybir.AluOpType.add,
                )
                # ---------- scoresT: sT[LK, 4] ----------
                ps_st = ps_oqs.tile([128, LT, 4], bf16, tag="oqs")
                for lt in range(LT):
                    nc.tensor.transpose(
                        ps_st[:, lt, :],
                        sc_s[0:4, lt * 128:(lt + 1) * 128],
                        ident[0:4, 0:4],
                    )
                st_s = scp_pool.tile([128, LT, 4], bf16, tag="st")
                nc.vector.tensor_copy(st_s[:, :, :], ps_st[:, :, :])

                for nq in range(NQ):
                    nqs = nq * QS
                    # ---------- qe_score quarter: qs[4, QS] ----------
                    qe_nat = qepool.tile([128, 4, LK], bf16, tag="qe_nat")
                    nc.gpsimd.dma_start(
                        out=qe_nat[:, :, :],
                        in_=qe_h[b, nqs:nqs + QS, :].rearrange(
                            "(t p) f -> p t f", p=128),
                    )
                    ps_qs = ps_oqs.tile([4, QS], f32, tag="oqs")
                    for lc in range(LT):
                        st_q = ps_stage.tile([128, QS], bf16, tag="stage")
                        for nt in range(4):
                            nc.tensor.transpose(
                                st_q[:, nt * 128:(nt + 1) * 128],
                                qe_nat[:, nt, lc * 128:(lc + 1) * 128],
                                ident[:, :],
                            )
                        qet_c = chunk_pool.tile([128, QS], bf16, tag="chunk")
                        _copy(lc, qet_c[:, :], st_q[:, :])
                        nc.tensor.matmul(
                            ps_qs[:, :], st_s[:, lc, :], qet_c[:, :],
                            start=(lc == 0), stop=(lc == LT - 1),
                        )
                    # for_correlation output (head 0)
                    fc_q = fcp_pool.tile([1, QS], f32, tag="fcq")
                    nc.vector.tensor_copy(fc_q[0:1, :], ps_qs[0:1, :])
                    nc.sync.dma_start(
                        out=fc_h[b, nqs:nqs + QS].rearrange("(o n) -> o n", o=1),
                        in_=fc_q[0:1, :],
                    )
                    # ---------- gate: g[4, QS] = sigmoid(lam*cw + lam1*qs) ----
                    cwl_q = gp_pool.tile([4, QS], f32, tag="cwl")
                    nc.gpsimd.dma_start(out=cwl_q[:, :],
                                        in_=cwl_h[b, :, nqs:nqs + QS])
                    g1 = gp_pool.tile([4, QS], f32, tag="g1")
                    nc.vector.tensor_scalar_mul(g1[:, :], ps_qs[:, :],
                                                lam1_s[0:4, 0:1])
                    nc.vector.tensor_add(g1[:, :], g1[:, :], cwl_q[:, :])
                    bcw = gp_pool.tile([4, QS], bf16, tag="bcw")
                    nc.scalar.activation(
                        bcw[:, :], g1[:, :],
                        mybir.ActivationFunctionType.Sigmoid,
                    )
                    # broadcast each head row to 128 partitions via selector MM
                    gate_rep = gate_pool.tile([128, H, QS], bf16, tag="gate")
                    for h in range(H):
                        ps_g = ps_oqs.tile([128, QS], f32, tag="oqs")
                        nc.tensor.matmul(
                            ps_g[:, :], sel_s[0:4, h * 128:(h + 1) * 128],
                            bcw[:, :], start=True, stop=True,
                        )
                        _copy(h, gate_rep[:, h, :], ps_g[:, :])
                    # ---------- v-projection quarter: vhT[D, QS] ----------
                    v_nat = vpool.tile([128, 4, F3], bf16, tag="v_nat")
                    nc.gpsimd.dma_start(
                        out=v_nat[:, :, :],
                        in_=v_h[b, nqs:nqs + QS, :].rearrange(
                            "(t p) f -> p t f", p=128),
                    )
                    ps_v = ps_vh.tile([128, MD, QS], f32, tag="vh")
                    for fc in range(FCV):
                        st_v = ps_stage.tile([128, QS], bf16, tag="stage")
                        for nt in range(4):
                            nc.tensor.transpose(
                                st_v[:, nt * 128:(nt + 1) * 128],
                                v_nat[:, nt, fc * 128:(fc + 1) * 128],
                                ident[:, :],
                            )
                        vt_c = chunk_pool.tile([128, QS], bf16, tag="chunk")
                        _copy(fc, vt_c[:, :], st_v[:, :])
                        for m in range(MD):
                            nc.tensor.matmul(
                                ps_v[:, m, :],
                                wv_s[:, fc, m * 128:(m + 1) * 128],
                                vt_c[:, :],
                                start=(fc == 0), stop=(fc == FCV - 1),
                            )
                    # ---------- gating -> gatedT (bf16) ----------
                    gatedT = gated_pool.tile([128, MD, QS], bf16, tag="gated")
                    for m in range(MD):
                        gt = gp_pool.tile([128, QS], bf16, tag="gt")
                        nc.scalar.activation(
                            gt[:, :], ps_v[:, m, :],
                            mybir.ActivationFunctionType.Identity,
                            bias=bv_s[:, m:m + 1],
                        )
                        nc.vector.tensor_mul(
                            gatedT[:, m, :], gt[:, :], gate_rep[:, m, :]
                        )
                    # ---------- out-projection quarter ----------
                    o_q = outst_pool.tile([128, 4, D], f32, tag="outst")
                    for nt in range(4):
                        ps_o = ps_oqs.tile([128, D], f32, tag="oqs")
                        for m in range(MD):
                            nc.tensor.matmul(
                                ps_o[:, :],
                                gatedT[:, m, nt * 128:(nt + 1) * 128],
                                wo_s[:, m, :],
                                start=(m == 0), stop=(m == MD - 1),
                            )
                        nc.vector.tensor_add(
                            o_q[:, nt, :], ps_o[:, :], bo_rep[:, :]
                        )
                    nc.sync.dma_start(
                        out=out_h[b, nqs:nqs + QS, :].rearrange(
                            "(t p) d -> p t d", p=128),
                        in_=o_q[:, :, :],
                    )
    nc.compile()
    return nc


def _host_prep(inputs):
    """Numpy-ify, compute the tiny q-side fold, build per-core input maps."""
    ins = {}
    for name, a in inputs.items():
        ins[name] = np.asarray(a)

    q = ins["q"].astype(np.float32).reshape(B, F2)
    Wq = ins["Wq"].astype(np.float32)
    bq = ins["bq"].astype(np.float32)
    Wk = ins["Wk"].astype(np.float32)
    bk = ins["bk"].astype(np.float32)

    qh = q @ Wq + bq                                   # [B, D]
    qh_r = qh.reshape(B, H, DK)                        # [B, H, DK]
    Wk_r = Wk.reshape(F2, H, DK)                       # [F2, H, DK]
    u = np.einsum("fhd,bhd->bfh", Wk_r, qh_r)          # [B, F2, H]
    c = np.einsum("hd,bhd->bh", bk.reshape(H, DK), qh_r)  # [B, H]

    lam = ins["lambdas"].astype(np.float32).reshape(H)
    cw = ins["correlation_weight"].astype(np.float32)
    cwl = lam[None, :, None] * cw[:, None, :]          # [B, H, N]
    lam1 = (1.0 - lam).astype(np.float32)
    sel = np.zeros((H, D), dtype=np.float32)
    for h in range(H):
        sel[h, h * DK:(h + 1) * DK] = 1.0

    in_maps = []
    for core in range(NCORES):
        s = slice(core * NB, (core + 1) * NB)
        in_maps.append({
            "k": np.ascontiguousarray(ins["k"][s], dtype=np.float32),
            "v": np.ascontiguousarray(ins["v_backward"][s], dtype=np.float32),
            "qe": np.ascontiguousarray(ins["qe"][s], dtype=np.float32),
            "cwl": np.ascontiguousarray(cwl[s]),
            "u": np.ascontiguousarray(u[s]),
            "c": np.ascontiguousarray(c[s]),
            "wv": np.ascontiguousarray(ins["Wv"], dtype=np.float32),
            "wo": np.ascontiguousarray(ins["Wo"], dtype=np.float32),
            "bv": np.ascontiguousarray(ins["bv"], dtype=np.float32),
            "bo": np.ascontiguousarray(ins["bo"], dtype=np.float32),
            "lam1": lam1,
            "sel": sel,
        })
    return in_maps


def kernel(**inputs):
    from concourse.bass_utils import run_bass_kernel_spmd

    if "nc" not in _cache:
        _cache["nc"] = _build_module()
    nc = _cache["nc"]

    in_maps = _host_prep(inputs)
    res = run_bass_kernel_spmd(nc, in_maps, list(range(NCORES)), trace=False)
    out = np.concatenate([res.results[i]["out"] for i in range(NCORES)], axis=0)
    fc = np.concatenate([res.results[i]["fc"] for i in range(NCORES)], axis=0)
    return out.astype(np.float32), fc.astype(np.float32)


# revision 12
# speedup vs baseline: 1.0525x; 1.0525x over previous
"""Trainium2 Bass kernel for nn_DBTKT_84164179132922 (dense_transformer).

Contract: kernel(**inputs) takes the FULL unsharded inputs (as produced by
setup_inputs()) and returns the full (output_backward, for_correlation_weight)
tuple. Internally shards the batch dim (32) across 8 NeuronCores (4 each).

Math (per batch b):
  qh   = q @ Wq + bq                          [512]
  u    = per-head fold of Wk with qh          [1024, 4]   (host, 0.04% of FLOPs)
  sc   = (k @ u + bk.qh) / sqrt(128)          [4, 1024]   (device)
  qs   = sc @ qe.T                            [4, 2048]
  fc   = qs[0]                                output 2
  g    = sigmoid(lam*cw + (1-lam)*qs)         [4, 2048]
  vhT  = Wv.T @ v.T  (+bv)                    [512, 2048]
  out  = (g ⊙ vhT).T-proj: gatedT.T @ Wo + bo [2048, 512]  output 1
"""

import sys

sys.path.insert(0, "/opt/trn_rl_repo")

import numpy as np

B, LK, N, D, H, DK = 32, 1024, 2048, 512, 4, 128
F2, F3 = 2 * D, 3 * D  # 1024, 1536
NCORES = 8
NB = B // NCORES  # batches per core
NQ = 4  # seq quarters of 512
QS = N // NQ  # 512

_cache = {}


def _patch_tile_drain():
    """This walrus build caps sync waits at 1 per non-EventSemaphore
    instruction; Tile's final drain can carry several. Split the extra waits
    onto bare drain instructions (same semantics: all waits happen before the
    end-of-kernel barrier + semaphore reset)."""
    import concourse.tile as tile
    import concourse.mybir as mybir
    from bass_rust import ScopedClock

    if getattr(tile.TileContext, "_drain_patch_applied", False):
        return

    def _drain_and_barrier(self, tick_clock, wait_clock):
        nc_ = self.nc
        drain_inst = nc_.sync.drain()
        wait_clock.add_sem_waits(
            drain_inst.ins, ScopedClock({None: tick_clock.global_clock})
        )
        si = drain_inst.ins.sync_info
        if si is not None and si.on_wait is not None and len(si.on_wait) > 1:
            waits = list(si.on_wait)
            drain_inst.ins.sync_info = mybir.SyncInfo(
                on_wait=waits[:1], on_update=list(si.on_update or [])
            )
            for w in waits[1:]:
                extra = nc_.sync.drain()
                esi = extra.ins.sync_info
                extra.ins.sync_info = mybir.SyncInfo(
                    on_wait=[w],
                    on_update=list(esi.on_update or []) if esi is not None else [],
                )
        nc_.all_engine_barrier()
        assert self.sems is not None
        popped = nc_._tile_sem_poison_stack.pop()
        assert popped is self._sem_poison
        nc_.clear_and_free_semaphores(list(self.sems.allocated().values()))
        nc_.all_engine_barrier()

    tile.TileContext._drain_and_barrier = _drain_and_barrier
    tile.TileContext._drain_patch_applied = True


def _patch_ldw_opt():
    """walrus --enable-ldw-opt=true rejects bass-emitted InstLdweights
    ("not compatible with LDW optimization") — keep the repo default."""
    return


def _build_module(nb=NB):
    import concourse.bass as bass
    import concourse.mybir as mybir
    import concourse.tile as tile
    from concourse import bacc, masks

    _patch_tile_drain()
    _patch_ldw_opt()

    f32 = mybir.dt.float32
    bf16 = mybir.dt.bfloat16
    SQ = float(1.0 / np.sqrt(DK))

    nc = bacc.Bacc("TRN2", target_bir_lowering=False, debug=False,
                   num_devices=NCORES)

    # ---- DRAM I/O ----
    k_h = nc.declare_dram_parameter("k", [nb, LK, F2], f32, isOutput=False)
    v_h = nc.declare_dram_parameter("v", [nb, N, F3], f32, isOutput=False)
    qe_h = nc.declare_dram_parameter("qe", [nb, N, LK], f32, isOutput=False)
    cwl_h = nc.declare_dram_parameter("cwl", [nb, H, N], f32, isOutput=False)
    u_h = nc.declare_dram_parameter("u", [nb, F2, H], f32, isOutput=False)
    c_h = nc.declare_dram_parameter("c", [nb, H], f32, isOutput=False)
    wv_h = nc.declare_dram_parameter("wv", [F3, D], f32, isOutput=False)
    wo_h = nc.declare_dram_parameter("wo", [D, D], f32, isOutput=False)
    bv_h = nc.declare_dram_parameter("bv", [D], f32, isOutput=False)
    bo_h = nc.declare_dram_parameter("bo", [D], f32, isOutput=False)
    lam1_h = nc.declare_dram_parameter("lam1", [H], f32, isOutput=False)
    sel_h = nc.declare_dram_parameter("sel", [H, D], f32, isOutput=False)

    out_h = nc.declare_dram_parameter("out", [nb, N, D], f32, isOutput=True)
    fc_h = nc.declare_dram_parameter("fc", [nb, N], f32, isOutput=True)

    FCK = F2 // 128  # 8 feature chunks for k
    FCV = F3 // 128  # 12 feature chunks for v
    LT = LK // 128   # 8 lk tiles
    MD = D // 128    # 4 dmodel chunks (== heads)

    with tile.TileContext(nc) as tc:
        with (
            tc.tile_pool(name="consts", bufs=1) as consts,
            tc.tile_pool(name="kpool", bufs=2) as kpool,
            tc.tile_pool(name="qepool", bufs=2) as qepool,
            tc.tile_pool(name="vpool", bufs=2) as vpool,
            tc.tile_pool(name="ktc", bufs=3) as ktc_pool,
            tc.tile_pool(name="chunk", bufs=4) as chunk_pool,
            tc.tile_pool(name="gate", bufs=2) as gate_pool,
            tc.tile_pool(name="gated", bufs=2) as gated_pool,
            tc.tile_pool(name="outst", bufs=2) as outst_pool,
            tc.tile_pool(name="scp", bufs=2) as scp_pool,
            tc.tile_pool(name="gp", bufs=2) as gp_pool,
            tc.tile_pool(name="fcp", bufs=2) as fcp_pool,
            tc.tile_pool(name="ps_stage", bufs=2, space="PSUM") as ps_stage,
            tc.tile_pool(name="ps_vh", bufs=1, space="PSUM") as ps_vh,
            tc.tile_pool(name="ps_oqs", bufs=2, space="PSUM") as ps_oqs,
        ):
            # ---- constants / weights (once) ----
            ident = consts.tile([128, 128], bf16)
            masks.make_identity(nc, ident[:, :])

            u_s = consts.tile([128, nb, FCK, H], bf16)
            nc.gpsimd.dma_start(
                out=u_s[:, :, :, :],
                in_=u_h.rearrange("b (c p) h -> p b c h", p=128),
            )
            c_s = consts.tile([H, nb], f32)
            nc.gpsimd.dma_start(out=c_s[:, :], in_=c_h.rearrange("b h -> h b"))
            bv_s = consts.tile([128, MD], f32)
            nc.gpsimd.dma_start(
                out=bv_s[:, :], in_=bv_h.rearrange("(m p) -> p m", p=128)
            )
            bo_rep = consts.tile([128, D], f32)
            bo_ap = bo_h.ap()
            nc.gpsimd.dma_start(
                out=bo_rep[:, :],
                in_=bass.AP(tensor=bo_ap.tensor, offset=bo_ap.offset,
                            ap=[[0, 128]] + list(bo_ap.ap)),
            )
            lam1_s = consts.tile([H, 1], f32)
            nc.gpsimd.dma_start(out=lam1_s[:, :],
                                in_=lam1_h.rearrange("(h o) -> h o", o=1))
            sel_s = consts.tile([H, D], bf16)
            nc.gpsimd.dma_start(out=sel_s[:, :], in_=sel_h[:, :])

            def _copy(i, dst, src):
                if i % 2 == 0:
                    nc.vector.tensor_copy(dst, src)
                else:
                    nc.scalar.copy(out=dst, in_=src)

            for b in range(nb):
                # ---------- scores: sc[4, LK] = (U.T @ kT) * SQ + c ----------
                k_nat = kpool.tile([128, LT, F2], bf16, tag="k_nat")
                nc.gpsimd.dma_start(
                    out=k_nat[:, :, :],
                    in_=k_h[b].rearrange("(t p) f -> p t f", p=128),
                )
                if b == 0:
                    wv_s = consts.tile([128, FCV, D], bf16)
                    nc.gpsimd.dma_start(
                        out=wv_s[:, :, :],
                        in_=wv_h.rearrange("(c p) d -> p c d", p=128),
                    )
                    wo_s = consts.tile([128, MD, D], bf16)
                    nc.gpsimd.dma_start(
                        out=wo_s[:, :, :],
                        in_=wo_h.rearrange("(c p) d -> p c d", p=128),
                    )
                ps_sc = ps_vh.tile([4, LK], f32, tag="vh")
                for fc in range(FCK):
                    st_k = ps_stage.tile([128, LK], bf16, tag="stage")
                    for lt in range(LT):
                        nc.tensor.transpose(
                            st_k[:, lt * 128:(lt + 1) * 128],
                            k_nat[:, lt, fc * 128:(fc + 1) * 128],
                            ident[:, :],
                        )
                    kt_c = ktc_pool.tile([128, LK], bf16, tag="ktc")
                    _copy(fc, kt_c[:, :], st_k[:, :])
                    for half in range(2):
                        nc.tensor.matmul(
                            ps_sc[:, half * 512:(half + 1) * 512],
                            u_s[:, b, fc, :],
                            kt_c[:, half * 512:(half + 1) * 512],
                            start=(fc == 0), stop=(fc == FCK - 1),
                        )
                # scale + bias -> bf16 scores
                sc_s = scp_pool.tile([4, LK], bf16, tag="sc")
                nc.vector.tensor_scalar(
                    out=sc_s[:, :], in0=ps_sc[:, :],
                    scalar1=SQ, scalar2=c_s[0:4, b:b + 1],
                    op0=mybir.AluOpType.mult, op1=mybir.AluOpType.add,
                )
                # ---------- scoresT: sT[LK, 4] ----------
                ps_st = ps_oqs.tile([128, LT, 4], bf16, tag="oqs")
                for lt in range(LT):
                    nc.tensor.transpose(
                        ps_st[:, lt, :],
                        sc_s[0:4, lt * 128:(lt + 1) * 128],
                        ident[0:4, 0:4],
                    )
                st_s = scp_pool.tile([128, LT, 4], bf16, tag="st")
                nc.vector.tensor_copy(st_s[:, :, :], ps_st[:, :, :])

                for nq in range(NQ):
                    nqs = nq * QS
                    # ---------- qe_score quarter: qs[4, QS] ----------
                    qe_nat = qepool.tile([128, 4, LK], bf16, tag="qe_nat")
                    nc.gpsimd.dma_start(
                        out=qe_nat[:, :, :],
                        in_=qe_h[b, nqs:nqs + QS, :].rearrange(
                            "(t p) f -> p t f", p=128),
                    )
                    ps_qs = ps_oqs.tile([4, QS], f32, tag="oqs")
                    for lc in range(LT):
                        st_q = ps_stage.tile([128, QS], bf16, tag="stage")
                        for nt in range(4):
                            nc.tensor.transpose(
                                st_q[:, nt * 128:(nt + 1) * 128],
                                qe_nat[:, nt, lc * 128:(lc + 1) * 128],
                                ident[:, :],
                            )
                        qet_c = chunk_pool.tile([128, QS], bf16, tag="chunk")
                        _copy(lc, qet_c[:, :], st_q[:, :])
                        nc.tensor.matmul(
                            ps_qs[:, :], st_s[:, lc, :], qet_c[:, :],
                            start=(lc == 0), stop=(lc == LT - 1),
                        )
                    # for_correlation output (head 0)
                    fc_q = fcp_pool.tile([1, QS], f32, tag="fcq")
                    nc.vector.tensor_copy(fc_q[0:1, :], ps_qs[0:1, :])
                    nc.sync.dma_start(
                        out=fc_h[b, nqs:nqs + QS].rearrange("(o n) -> o n", o=1),
                        in_=fc_q[0:1, :],
                    )
                    # ---------- gate: g[4, QS] = sigmoid(lam*cw + lam1*qs) ----
                    cwl_q = gp_pool.tile([4, QS], f32, tag="cwl")
                    nc.gpsimd.dma_start(out=cwl_q[:, :],
                                        in_=cwl_h[b, :, nqs:nqs + QS])
                    g1 = gp_pool.tile([4, QS], f32, tag="g1")
                    nc.vector.tensor_scalar_mul(g1[:, :], ps_qs[:, :],
                                                lam1_s[0:4, 0:1])
                    nc.vector.tensor_add(g1[:, :], g1[:, :], cwl_q[:, :])
                    bcw = gp_pool.tile([4, QS], bf16, tag="bcw")
                    nc.scalar.activation(
                        bcw[:, :], g1[:, :],
                        mybir.ActivationFunctionType.Sigmoid,
                    )
                    # broadcast each head row to 128 partitions via selector MM
                    gate_rep = gate_pool.tile([128, H, QS], bf16, tag="gate")
                    for h in range(H):
                        ps_g = ps_oqs.tile([128, QS], f32, tag="oqs")
                        nc.tensor.matmul(
                            ps_g[:, :], sel_s[0:4, h * 128:(h + 1) * 128],
                            bcw[:, :], start=True, stop=True,
                        )
                        _copy(h, gate_rep[:, h, :], ps_g[:, :])
                    # ---------- v-projection quarter: vhT[D, QS] ----------
                    v_nat = vpool.tile([128, 4, F3], bf16, tag="v_nat")
                    nc.gpsimd.dma_start(
                        out=v_nat[:, :, :],
                        in_=v_h[b, nqs:nqs + QS, :].rearrange(
                            "(t p) f -> p t f", p=128),
                    )
                    ps_v = ps_vh.tile([128, MD, QS], f32, tag="vh")
                    for fc in range(FCV):
                        st_v = ps_stage.tile([128, QS], bf16, tag="stage")
                        for nt in range(4):
                            nc.tensor.transpose(
                                st_v[:, nt * 128:(nt + 1) * 128],
                                v_nat[:, nt, fc * 128:(fc + 1) * 128],
                                ident[:, :],
                            )
                        vt_c = chunk_pool.tile([128, QS], bf16, tag="chunk")
                        _copy(fc, vt_c[:, :], st_v[:, :])
                        for m in range(MD):
                            nc.tensor.matmul(
                                ps_v[:, m, :],
                                wv_s[:, fc, m * 128:(m + 1) * 128],
                                vt_c[:, :],
                                start=(fc == 0), stop=(fc == FCV - 1),
                            )
                    # ---------- gating -> gatedT (bf16) ----------
                    gatedT = gated_pool.tile([128, MD, QS], bf16, tag="gated")
                    for m in range(MD):
                        gt = gp_pool.tile([128, QS], bf16, tag="gt")
                        nc.scalar.activation(
                            gt[:, :], ps_v[:, m, :],
                            mybir.ActivationFunctionType.Identity,
                            bias=bv_s[:, m:m + 1],
                        )
                        nc.vector.tensor_mul(
                            gatedT[:, m, :], gt[:, :], gate_rep[:, m, :]
                        )
                    # ---------- out-projection quarter ----------
                    o_q = outst_pool.tile([128, 4, D], f32, tag="outst")
                    for nt in range(4):
                        ps_o = ps_oqs.tile([128, D], f32, tag="oqs")
                        for m in range(MD):
                            nc.tensor.matmul(
                                ps_o[:, :],
                                gatedT[:, m, nt * 128:(nt + 1) * 128],
                                wo_s[:, m, :],
                                start=(m == 0), stop=(m == MD - 1),
                            )
                        nc.vector.tensor_add(
                            o_q[:, nt, :], ps_o[:, :], bo_rep[:, :]
                        )
                    nc.sync.dma_start(
                        out=out_h[b, nqs:nqs + QS, :].rearrange(
                            "(t p) d -> p t d", p=128),
                        in_=o_q[:, :, :],
                    )
    nc.compile()
    return nc


def _host_prep(inputs):
    """Numpy-ify, compute the tiny q-side fold, build per-core input maps."""
    ins = {}
    for name, a in inputs.items():
        ins[name] = np.asarray(a)

    q = ins["q"].astype(np.float32).reshape(B, F2)
    Wq = ins["Wq"].astype(np.float32)
    bq = ins["bq"].astype(np.float32)
    Wk = ins["Wk"].astype(np.float32)
    bk = ins["bk"].astype(np.float32)

    qh = q @ Wq + bq                                   # [B, D]
    qh_r = qh.reshape(B, H, DK)                        # [B, H, DK]
    Wk_r = Wk.reshape(F2, H, DK)                       # [F2, H, DK]
    u = np.einsum("fhd,bhd->bfh", Wk_r, qh_r)          # [B, F2, H]
    c = np.einsum("hd,bhd->bh", bk.reshape(H, DK), qh_r)  # [B, H]

    lam = ins["lambdas"].astype(np.float32).reshape(H)
    cw = ins["correlation_weight"].astype(np.float32)
    cwl = lam[None, :, None] * cw[:, None, :]          # [B, H, N]
    lam1 = (1.0 - lam).astype(np.float32)
    sel = np.zeros((H, D), dtype=np.float32)
    for h in range(H):
        sel[h, h * DK:(h + 1) * DK] = 1.0

    in_maps = []
    for core in range(NCORES):
        s = slice(core * NB, (core + 1) * NB)
        in_maps.append({
            "k": np.ascontiguousarray(ins["k"][s], dtype=np.float32),
            "v": np.ascontiguousarray(ins["v_backward"][s], dtype=np.float32),
            "qe": np.ascontiguousarray(ins["qe"][s], dtype=np.float32),
            "cwl": np.ascontiguousarray(cwl[s]),
            "u": np.ascontiguousarray(u[s]),
            "c": np.ascontiguousarray(c[s]),
            "wv": np.ascontiguousarray(ins["Wv"], dtype=np.float32),
            "wo": np.ascontiguousarray(ins["Wo"], dtype=np.float32),
            "bv": np.ascontiguousarray(ins["bv"], dtype=np.float32),
            "bo": np.ascontiguousarray(ins["bo"], dtype=np.float32),
            "lam1": lam1,
            "sel": sel,
        })
    return in_maps


def kernel(**inputs):
    from concourse.bass_utils import run_bass_kernel_spmd

    if "nc" not in _cache:
        _cache["nc"] = _build_module()
    nc = _cache["nc"]

    in_maps = _host_prep(inputs)
    res = run_bass_kernel_spmd(nc, in_maps, list(range(NCORES)), trace=False)
    out = np.concatenate([res.results[i]["out"] for i in range(NCORES)], axis=0)
    fc = np.concatenate([res.results[i]["fc"] for i in range(NCORES)], axis=0)
    return out.astype(np.float32), fc.astype(np.float32)


# revision 13
# speedup vs baseline: 1.1505x; 1.0931x over previous
"""Trainium2 Bass kernel for nn_DBTKT_84164179132922 (dense_transformer).

Contract: kernel(**inputs) takes the FULL unsharded inputs (as produced by
setup_inputs()) and returns the full (output_backward, for_correlation_weight)
tuple. Internally shards the batch dim (32) across 8 NeuronCores (4 each).

Math (per batch b):
  qh   = q @ Wq + bq                          [512]
  u    = per-head fold of Wk with qh          [1024, 4]   (host, 0.04% of FLOPs)
  sc   = (k @ u + bk.qh) / sqrt(128)          [4, 1024]   (device)
  qs   = sc @ qe.T                            [4, 2048]
  fc   = qs[0]                                output 2
  g    = sigmoid(lam*cw + (1-lam)*qs)         [4, 2048]
  vhT  = Wv.T @ v.T  (+bv)                    [512, 2048]
  out  = (g ⊙ vhT).T-proj: gatedT.T @ Wo + bo [2048, 512]  output 1
"""

import sys

sys.path.insert(0, "/opt/trn_rl_repo")

import numpy as np

B, LK, N, D, H, DK = 32, 1024, 2048, 512, 4, 128
F2, F3 = 2 * D, 3 * D  # 1024, 1536
NCORES = 8
NB = B // NCORES  # batches per core
NQ = 4  # seq quarters of 512
QS = N // NQ  # 512

_cache = {}


def _patch_tile_drain():
    """This walrus build caps sync waits at 1 per non-EventSemaphore
    instruction; Tile's final drain can carry several. Split the extra waits
    onto bare drain instructions (same semantics: all waits happen before the
    end-of-kernel barrier + semaphore reset)."""
    import concourse.tile as tile
    import concourse.mybir as mybir
    from bass_rust import ScopedClock

    if getattr(tile.TileContext, "_drain_patch_applied", False):
        return

    def _drain_and_barrier(self, tick_clock, wait_clock):
        nc_ = self.nc
        drain_inst = nc_.sync.drain()
        wait_clock.add_sem_waits(
            drain_inst.ins, ScopedClock({None: tick_clock.global_clock})
        )
        si = drain_inst.ins.sync_info
        if si is not None and si.on_wait is not None and len(si.on_wait) > 1:
            waits = list(si.on_wait)
            drain_inst.ins.sync_info = mybir.SyncInfo(
                on_wait=waits[:1], on_update=list(si.on_update or [])
            )
            for w in waits[1:]:
                extra = nc_.sync.drain()
                esi = extra.ins.sync_info
                extra.ins.sync_info = mybir.SyncInfo(
                    on_wait=[w],
                    on_update=list(esi.on_update or []) if esi is not None else [],
                )
        nc_.all_engine_barrier()
        assert self.sems is not None
        popped = nc_._tile_sem_poison_stack.pop()
        assert popped is self._sem_poison
        nc_.clear_and_free_semaphores(list(self.sems.allocated().values()))
        nc_.all_engine_barrier()

    tile.TileContext._drain_and_barrier = _drain_and_barrier
    tile.TileContext._drain_patch_applied = True


def _patch_ldw_opt():
    """walrus --enable-ldw-opt=true rejects bass-emitted InstLdweights
    ("not compatible with LDW optimization") — keep the repo default."""
    return


def _build_module(nb=NB):
    import concourse.bass as bass
    import concourse.mybir as mybir
    import concourse.tile as tile
    from concourse import bacc, masks

    _patch_tile_drain()
    _patch_ldw_opt()

    f32 = mybir.dt.float32
    bf16 = mybir.dt.bfloat16
    SQ = float(1.0 / np.sqrt(DK))

    nc = bacc.Bacc("TRN2", target_bir_lowering=False, debug=False,
                   num_devices=NCORES)

    # ---- DRAM I/O ----
    k_h = nc.declare_dram_parameter("k", [nb, LK, F2], f32, isOutput=False)
    v_h = nc.declare_dram_parameter("v", [nb, N, F3], f32, isOutput=False)
    qe_h = nc.declare_dram_parameter("qe", [nb, N, LK], f32, isOutput=False)
    cwl_h = nc.declare_dram_parameter("cwl", [nb, H, N], f32, isOutput=False)
    u_h = nc.declare_dram_parameter("u", [nb, F2, H], f32, isOutput=False)
    c_h = nc.declare_dram_parameter("c", [nb, H], f32, isOutput=False)
    wv_h = nc.declare_dram_parameter("wv", [F3, D], f32, isOutput=False)
    wo_h = nc.declare_dram_parameter("wo", [D, D], f32, isOutput=False)
    bv_h = nc.declare_dram_parameter("bv", [D], f32, isOutput=False)
    bo_h = nc.declare_dram_parameter("bo", [D], f32, isOutput=False)
    lam1_h = nc.declare_dram_parameter("lam1", [H], f32, isOutput=False)
    sel_h = nc.declare_dram_parameter("sel", [H, D], f32, isOutput=False)

    out_h = nc.declare_dram_parameter("out", [nb, N, D], f32, isOutput=True)
    fc_h = nc.declare_dram_parameter("fc", [nb, N], f32, isOutput=True)

    FCK = F2 // 128  # 8 feature chunks for k
    FCV = F3 // 128  # 12 feature chunks for v
    LT = LK // 128   # 8 lk tiles
    MD = D // 128    # 4 dmodel chunks (== heads)

    with tile.TileContext(nc) as tc:
        with (
            tc.tile_pool(name="consts", bufs=1) as consts,
            tc.tile_pool(name="kpool", bufs=2) as kpool,
            tc.tile_pool(name="qepool", bufs=2) as qepool,
            tc.tile_pool(name="vpool", bufs=2) as vpool,
            tc.tile_pool(name="ktc", bufs=3) as ktc_pool,
            tc.tile_pool(name="chunk", bufs=4) as chunk_pool,
            tc.tile_pool(name="gate", bufs=2) as gate_pool,
            tc.tile_pool(name="gated", bufs=2) as gated_pool,
            tc.tile_pool(name="outst", bufs=2) as outst_pool,
            tc.tile_pool(name="scp", bufs=2) as scp_pool,
            tc.tile_pool(name="gp", bufs=2) as gp_pool,
            tc.tile_pool(name="fcp", bufs=2) as fcp_pool,
            tc.tile_pool(name="ps_stage", bufs=2, space="PSUM") as ps_stage,
            tc.tile_pool(name="ps_vh", bufs=1, space="PSUM") as ps_vh,
            tc.tile_pool(name="ps_oqs", bufs=2, space="PSUM") as ps_oqs,
        ):
            # ---- constants / weights (once) ----
            ident = consts.tile([128, 128], bf16)
            masks.make_identity(nc, ident[:, :])

            u_s = consts.tile([128, nb, FCK, H], bf16)
            nc.gpsimd.dma_start(
                out=u_s[:, :, :, :],
                in_=u_h.rearrange("b (c p) h -> p b c h", p=128),
            )
            c_s = consts.tile([H, nb], f32)
            nc.gpsimd.dma_start(out=c_s[:, :], in_=c_h.rearrange("b h -> h b"))
            bv_s = consts.tile([128, MD], f32)
            nc.gpsimd.dma_start(
                out=bv_s[:, :], in_=bv_h.rearrange("(m p) -> p m", p=128)
            )
            bo_rep = consts.tile([128, D], f32)
            bo_ap = bo_h.ap()
            nc.gpsimd.dma_start(
                out=bo_rep[:, :],
                in_=bass.AP(tensor=bo_ap.tensor, offset=bo_ap.offset,
                            ap=[[0, 128]] + list(bo_ap.ap)),
            )
            lam1_s = consts.tile([H, 1], f32)
            nc.gpsimd.dma_start(out=lam1_s[:, :],
                                in_=lam1_h.rearrange("(h o) -> h o", o=1))
            sel_s = consts.tile([H, D], bf16)
            nc.gpsimd.dma_start(out=sel_s[:, :], in_=sel_h[:, :])

            def _copy(i, dst, src):
                if i % 2 == 0:
                    nc.vector.tensor_copy(dst, src)
                else:
                    nc.scalar.copy(out=dst, in_=src)

            for b in range(nb):
                # ---------- scores: sc[4, LK] = (U.T @ kT) * SQ + c ----------
                k_nat = kpool.tile([128, LT, F2], bf16, tag="k_nat")
                nc.gpsimd.dma_start(
                    out=k_nat[:, :, :],
                    in_=k_h[b].rearrange("(t p) f -> p t f", p=128),
                )
                if b == 0:
                    wv_s = consts.tile([128, FCV, D], bf16)
                    nc.gpsimd.dma_start(
                        out=wv_s[:, :, :],
                        in_=wv_h.rearrange("(c p) d -> p c d", p=128),
                    )
                    wo_s = consts.tile([128, MD, D], bf16)
                    nc.gpsimd.dma_start(
                        out=wo_s[:, :, :],
                        in_=wo_h.rearrange("(c p) d -> p c d", p=128),
                    )
                ps_sc = ps_vh.tile([4, LK], f32, tag="vh")
                for fc in range(FCK):
                    st_k = ps_stage.tile([128, LK], bf16, tag="stage")
                    for lt in range(LT):
                        nc.tensor.transpose(
                            st_k[:, lt * 128:(lt + 1) * 128],
                            k_nat[:, lt, fc * 128:(fc + 1) * 128],
                            ident[:, :],
                        )
                    kt_c = ktc_pool.tile([128, LK], bf16, tag="ktc")
                    _copy(fc, kt_c[:, :], st_k[:, :])
                    for half in range(2):
                        nc.tensor.matmul(
                            ps_sc[:, half * 512:(half + 1) * 512],
                            u_s[:, b, fc, :],
                            kt_c[:, half * 512:(half + 1) * 512],
                            start=(fc == 0), stop=(fc == FCK - 1),
                        )
                # scale + bias -> bf16 scores
                sc_s = scp_pool.tile([4, LK], bf16, tag="sc")
                nc.vector.tensor_scalar(
                    out=sc_s[:, :], in0=ps_sc[:, :],
                    scalar1=SQ, scalar2=c_s[0:4, b:b + 1],
                    op0=mybir.AluOpType.mult, op1=mybir.AluOpType.add,
                )
                # ---------- scoresT: sT[LK, 4] ----------
                ps_st = ps_oqs.tile([128, LT, 4], bf16, tag="oqs")
                for lt in range(LT):
                    nc.tensor.transpose(
                        ps_st[:, lt, :],
                        sc_s[0:4, lt * 128:(lt + 1) * 128],
                        ident[0:4, 0:4],
                    )
                st_s = scp_pool.tile([128, LT, 4], bf16, tag="st")
                nc.vector.tensor_copy(st_s[:, :, :], ps_st[:, :, :])

                for nq in range(NQ):
                    nqs = nq * QS
                    # ---------- qe_score quarter: qs[4, QS] ----------
                    qe_nat = qepool.tile([128, 4, LK], bf16, tag="qe_nat")
                    nc.gpsimd.dma_start(
                        out=qe_nat[:, :, :],
                        in_=qe_h[b, nqs:nqs + QS, :].rearrange(
                            "(t p) f -> p t f", p=128),
                    )
                    ps_qs = ps_oqs.tile([4, QS], f32, tag="oqs")
                    for lcp in range(LT // 2):
                        st_q = ps_stage.tile([128, 2, QS], bf16, tag="stage")
                        for j in range(2):
                            lc = 2 * lcp + j
                            for nt in range(4):
                                nc.tensor.transpose(
                                    st_q[:, j, nt * 128:(nt + 1) * 128],
                                    qe_nat[:, nt, lc * 128:(lc + 1) * 128],
                                    ident[:, :],
                                )
                        qet_c = chunk_pool.tile([128, 2, QS], bf16, tag="chunk")
                        _copy(lcp, qet_c[:, :, :], st_q[:, :, :])
                        for j in range(2):
                            lc = 2 * lcp + j
                            nc.tensor.matmul(
                                ps_qs[:, :], st_s[:, lc, :], qet_c[:, j, :],
                                start=(lc == 0), stop=(lc == LT - 1),
                            )
                    # for_correlation output (head 0)
                    fc_q = fcp_pool.tile([1, QS], f32, tag="fcq")
                    nc.vector.tensor_copy(fc_q[0:1, :], ps_qs[0:1, :])
                    nc.sync.dma_start(
                        out=fc_h[b, nqs:nqs + QS].rearrange("(o n) -> o n", o=1),
                        in_=fc_q[0:1, :],
                    )
                    # ---------- gate: g[4, QS] = sigmoid(lam*cw + lam1*qs) ----
                    cwl_q = gp_pool.tile([4, QS], f32, tag="cwl")
                    nc.gpsimd.dma_start(out=cwl_q[:, :],
                                        in_=cwl_h[b, :, nqs:nqs + QS])
                    g1 = gp_pool.tile([4, QS], f32, tag="g1")
                    nc.vector.tensor_scalar_mul(g1[:, :], ps_qs[:, :],
                                                lam1_s[0:4, 0:1])
                    nc.vector.tensor_add(g1[:, :], g1[:, :], cwl_q[:, :])
                    bcw = gp_pool.tile([4, QS], bf16, tag="bcw")
                    nc.scalar.activation(
                        bcw[:, :], g1[:, :],
                        mybir.ActivationFunctionType.Sigmoid,
                    )
                    # broadcast each head row to 128 partitions via selector MM
                    gate_rep = gate_pool.tile([128, H, QS], bf16, tag="gate")
                    for h in range(H):
                        ps_g = ps_oqs.tile([128, QS], f32, tag="oqs")
                        nc.tensor.matmul(
                            ps_g[:, :], sel_s[0:4, h * 128:(h + 1) * 128],
                            bcw[:, :], start=True, stop=True,
                        )
                        _copy(h, gate_rep[:, h, :], ps_g[:, :])
                    # ---------- v-projection quarter: vhT[D, QS] ----------
                    v_nat = vpool.tile([128, 4, F3], bf16, tag="v_nat")
                    nc.gpsimd.dma_start(
                        out=v_nat[:, :, :],
                        in_=v_h[b, nqs:nqs + QS, :].rearrange(
                            "(t p) f -> p t f", p=128),
                    )
                    ps_v = ps_vh.tile([128, MD, QS], f32, tag="vh")
                    for fcp in range(FCV // 2):
                        st_v = ps_stage.tile([128, 2, QS], bf16, tag="stage")
                        for j in range(2):
                            fc = 2 * fcp + j
                            for nt in range(4):
                                nc.tensor.transpose(
                                    st_v[:, j, nt * 128:(nt + 1) * 128],
                                    v_nat[:, nt, fc * 128:(fc + 1) * 128],
                                    ident[:, :],
                                )
                        vt_c = chunk_pool.tile([128, 2, QS], bf16, tag="chunk")
                        _copy(fcp, vt_c[:, :, :], st_v[:, :, :])
                        for j in range(2):
                            fc = 2 * fcp + j
                            for m in range(MD):
                                nc.tensor.matmul(
                                    ps_v[:, m, :],
                                    wv_s[:, fc, m * 128:(m + 1) * 128],
                                    vt_c[:, j, :],
                                    start=(fc == 0), stop=(fc == FCV - 1),
                                )
                    # ---------- gating -> gatedT (bf16) ----------
                    gatedT = gated_pool.tile([128, MD, QS], bf16, tag="gated")
                    for m in range(MD):
                        gt = gp_pool.tile([128, QS], bf16, tag="gt")
                        nc.scalar.activation(
                            gt[:, :], ps_v[:, m, :],
                            mybir.ActivationFunctionType.Identity,
                            bias=bv_s[:, m:m + 1],
                        )
                        nc.vector.tensor_mul(
                            gatedT[:, m, :], gt[:, :], gate_rep[:, m, :]
                        )
                    # ---------- out-projection quarter ----------
                    o_q = outst_pool.tile([128, 4, D], f32, tag="outst")
                    for nt in range(4):
                        ps_o = ps_oqs.tile([128, D], f32, tag="oqs")
                        for m in range(MD):
                            nc.tensor.matmul(
                                ps_o[:, :],
                                gatedT[:, m, nt * 128:(nt + 1) * 128],
                                wo_s[:, m, :],
                                start=(m == 0), stop=(m == MD - 1),
                            )
                        nc.vector.tensor_add(
                            o_q[:, nt, :], ps_o[:, :], bo_rep[:, :]
                        )
                    nc.sync.dma_start(
                        out=out_h[b, nqs:nqs + QS, :].rearrange(
                            "(t p) d -> p t d", p=128),
                        in_=o_q[:, :, :],
                    )
    nc.compile()
    return nc


def _host_prep(inputs):
    """Numpy-ify, compute the tiny q-side fold, build per-core input maps."""
    ins = {}
    for name, a in inputs.items():
        ins[name] = np.asarray(a)

    q = ins["q"].astype(np.float32).reshape(B, F2)
    Wq = ins["Wq"].astype(np.float32)
    bq = ins["bq"].astype(np.float32)
    Wk = ins["Wk"].astype(np.float32)
    bk = ins["bk"].astype(np.float32)

    qh = q @ Wq + bq                                   # [B, D]
    qh_r = qh.reshape(B, H, DK)                        # [B, H, DK]
    Wk_r = Wk.reshape(F2, H, DK)                       # [F2, H, DK]
    u = np.einsum("fhd,bhd->bfh", Wk_r, qh_r)          # [B, F2, H]
    c = np.einsum("hd,bhd->bh", bk.reshape(H, DK), qh_r)  # [B, H]

    lam = ins["lambdas"].astype(np.float32).reshape(H)
    cw = ins["correlation_weight"].astype(np.float32)
    cwl = lam[None, :, None] * cw[:, None, :]          # [B, H, N]
    lam1 = (1.0 - lam).astype(np.float32)
    sel = np.zeros((H, D), dtype=np.float32)
    for h in range(H):
        sel[h, h * DK:(h + 1) * DK] = 1.0

    in_maps = []
    for core in range(NCORES):
        s = slice(core * NB, (core + 1) * NB)
        in_maps.append({
            "k": np.ascontiguousarray(ins["k"][s], dtype=np.float32),
            "v": np.ascontiguousarray(ins["v_backward"][s], dtype=np.float32),
            "qe": np.ascontiguousarray(ins["qe"][s], dtype=np.float32),
            "cwl": np.ascontiguousarray(cwl[s]),
            "u": np.ascontiguousarray(u[s]),
            "c": np.ascontiguousarray(c[s]),
            "wv": np.ascontiguousarray(ins["Wv"], dtype=np.float32),
            "wo": np.ascontiguousarray(ins["Wo"], dtype=np.float32),
            "bv": np.ascontiguousarray(ins["bv"], dtype=np.float32),
            "bo": np.ascontiguousarray(ins["bo"], dtype=np.float32),
            "lam1": lam1,
            "sel": sel,
        })
    return in_maps


def kernel(**inputs):
    from concourse.bass_utils import run_bass_kernel_spmd

    if "nc" not in _cache:
        _cache["nc"] = _build_module()
    nc = _cache["nc"]

    in_maps = _host_prep(inputs)
    res = run_bass_kernel_spmd(nc, in_maps, list(range(NCORES)), trace=False)
    out = np.concatenate([res.results[i]["out"] for i in range(NCORES)], axis=0)
    fc = np.concatenate([res.results[i]["fc"] for i in range(NCORES)], axis=0)
    return out.astype(np.float32), fc.astype(np.float32)
